# revision 23
# baseline (speedup 1.0000x reference)
"""Fused transformer block (B=4, N=1024, C=768, H=12, HID=3072) on 8 TRN2
NeuronCores.

Sharding: data-parallel over (batch, seq-half): core c handles batch c//2,
sequence half c%2 -> 512 query rows. Keys/values are computed per core from a
host-side COMPACTED key list: the attention mask zeroes ~half the keys and
their softmax weight in the reference is exactly 0 (exp(-10000) underflows),
so masked keys are dropped on the host and the key set padded to a multiple
of 128 (pad rows are zero, their v/denominator columns are masked to 0).

The kernel is PE-sequencer-issue-bound, so the design minimizes matmul
instruction count:
  - fp8(e4m3) DoubleRow matmuls (contraction 256/instruction) for qkv, av,
    proj and fc1; weights are scaled x32 on the host into fp8's sweet spot
    and the 1/32 factors folded into activation scales / epilogues.
  - transposes run on the DMA xbar (bf16), not the PE.
  - scores and fc2 stay bf16 for accuracy; softmax normalizer comes from an
    extra ones-column in v (col 64 of each head block), divided out on DVE.
  - LN rstd = exp(-0.5*ln(var+eps)) so the Act engine only ever needs the
    natural_log_exp table plus one swap to the gelu table for fc1.
Vector work is split DVE (bn_stats, divides, psum->bf16 copies) vs Pool
(LN apply, fp8 casts, v_aug packing, residual epilogues).
"""

import numpy as np
import ml_dtypes

import concourse.bass as bass
import concourse.bacc as bacc
import concourse.mybir as mybir
import concourse.tile as tile
from concourse.bass_utils import run_bass_kernel_spmd

P = 128
DIM = 768
HEADS = 12
HD = 64
HID = 3072
EPS = 1e-5
NT_O = 4  # token tiles for the core's own 512 rows
KC = DIM // P  # 6
KH = HID // P  # 24
N_CORES = 8
SW = 32.0  # host-side fp8 weight scale

bf16 = mybir.dt.bfloat16
fp8 = mybir.dt.float8e4
f32 = mybir.dt.float32
ALU = mybir.AluOpType
ACT_F = mybir.ActivationFunctionType
DR = mybir.MatmulPerfMode.DoubleRow

_PHASE_MARKS = []  # (phase_name, first_instruction_number); for sim analysis


def _mark(nc, phase):
    _PHASE_MARKS.append((phase, int(nc.get_next_instruction_name().split("-")[1])))


def _ln_stats(nc, lnp, x_ap, mv_g, slot):
    """bn stats for one [128,768] tile -> mean/var into mv_g[:, slot, :]."""
    stats = lnp.tile([P, 3, 6], f32, tag="ln_stats")
    xg = x_ap.rearrange("p (s d) -> p s d", s=3)
    for s in range(3):
        nc.vector.bn_stats(out=stats[:, s, :], in_=xg[:, s, :])
    nc.vector.bn_aggr(out=mv_g[:, slot, :], in_=stats)


def _ln_rstd_group(nc, lnp, mv_g, eps_t, n):
    """rstd for a group of n LN tiles in 2 Act ops: exp(-0.5*ln(var+eps)).
    Both funcs live in the natural_log_exp table (same one the softmax exp
    uses), so no activation-table swaps are ever needed outside gelu."""
    lnv = lnp.tile([P, n], f32, tag="ln_lnv")
    nc.scalar.activation(
        out=lnv, in_=mv_g[:, 0:n, 1], func=ACT_F.Ln, bias=eps_t, scale=1.0
    )
    rstd_g = lnp.tile([P, n], f32, tag="ln_rstd")
    nc.scalar.activation(out=rstd_g, in_=lnv, func=ACT_F.Exp, bias=0.0, scale=-0.5)
    return rstd_g


def _ln_apply(nc, x_ap, out_ap, mv_g, rstd_g, slot, g_rep, b_rep):
    nc.vector.tensor_scalar(
        out=out_ap, in0=x_ap, scalar1=mv_g[:, slot, 0:1],
        scalar2=rstd_g[:, slot : slot + 1], op0=ALU.subtract, op1=ALU.mult,
    )
    if g_rep is not None:
        nc.gpsimd.tensor_mul(out=out_ap, in0=out_ap, in1=g_rep)
    if b_rep is not None:
        nc.gpsimd.tensor_add(out=out_ap, in0=out_ap, in1=b_rep)


def _dr_steps(nkt):
    """(pairs, has_tail) for a DoubleRow contraction over nkt 128-tiles."""
    return nkt // 2, nkt % 2


def _build(flags, repeat=1):
    nc = bacc.Bacc(None)
    _PHASE_MARKS.clear()
    NKT = flags["nkt"]
    NK = NKT * P
    fc1_fp8 = flags["fc1_fp8"]
    fc2_fp8 = flags["fc2_fp8"]

    xp_e = nc.declare_dram_parameter("xp", [512, DIM], f32, isOutput=False)
    xk_e = nc.declare_dram_parameter("xk", [NK, DIM], bf16, isOutput=False)
    m01k_e = nc.declare_dram_parameter("m01k", [P, NKT], f32, isOutput=False)
    wqk_e = nc.declare_dram_parameter("wqk", [DIM, 2 * DIM], fp8, isOutput=False)
    wv_e = nc.declare_dram_parameter("wv", [DIM, DIM], fp8, isOutput=False)
    wp_e = nc.declare_dram_parameter("wp", [DIM, DIM], fp8, isOutput=False)
    wf1_e = nc.declare_dram_parameter(
        "wf1", [DIM, HID], fp8 if fc1_fp8 else bf16, isOutput=False
    )
    wf2_e = nc.declare_dram_parameter(
        "wf2", [HID, DIM], fp8 if fc2_fp8 else bf16, isOutput=False
    )
    y_e = nc.declare_dram_parameter("y", [512, DIM], f32, isOutput=True)

    opt = {}
    for name, dim, fl in (
        ("ln1g", DIM, "ln1_gb"), ("ln1b", DIM, "ln1_gb"),
        ("ln2g", DIM, "ln2_gb"), ("ln2b", DIM, "ln2_gb"),
        ("bqk", 2 * DIM, "bqk"), ("bv", DIM, "bv"), ("bp", DIM, "bp"),
        ("bf1", HID, "bf1"), ("bf2", DIM, "bf2"),
    ):
        if flags[fl]:
            opt[name] = nc.declare_dram_parameter(name, [dim], f32, isOutput=False)

    def bcast(ap):
        # replicate a [D] DRAM vector across all 128 partitions for DMA
        return bass.AP(tensor=ap.tensor, offset=ap.offset, ap=[[0, P], *ap.ap])

    with tile.TileContext(nc) as tc:
        import contextlib

        with contextlib.ExitStack() as ctx:
            singles = ctx.enter_context(tc.tile_pool(name="singles", bufs=1))
            lnp = ctx.enter_context(tc.tile_pool(name="ln", bufs=4))
            htmp = ctx.enter_context(tc.tile_pool(name="htmp", bufs=3))
            ttmp = ctx.enter_context(tc.tile_pool(name="ttmp", bufs=3))
            big = ctx.enter_context(tc.tile_pool(name="big", bufs=1))
            ppool = ctx.enter_context(tc.tile_pool(name="pT", bufs=3))
            mmps = ctx.enter_context(tc.tile_pool(name="mmps", bufs=4, space="PSUM"))
            sps = ctx.enter_context(tc.tile_pool(name="sps", bufs=2, space="PSUM"))

            # --- constants ---
            eps_t = singles.tile([P, 1], f32)
            nc.vector.memset(eps_t, EPS)
            m01k_sb = singles.tile([P, NKT], f32)
            nc.sync.dma_start(out=m01k_sb, in_=m01k_e[:, :])

            reps = {}
            for name in ("ln1g", "ln1b", "ln2g", "ln2b", "bv", "bp", "bf2"):
                if name in opt:
                    r = singles.tile([P, opt[name].shape[0]], f32, tag=name)
                    nc.sync.dma_start(out=r, in_=bcast(opt[name][:]))
                    reps[name] = r
            bqk_sb = bf1_sb = None
            if "bqk" in opt:
                bqk_sb = singles.tile([P, 2 * KC], f32, tag="bqk")
                nc.sync.dma_start(
                    out=bqk_sb, in_=opt["bqk"][:].rearrange("(t p) -> p t", p=P)
                )
            if "bf1" in opt:
                bf1_sb = singles.tile([P, KH], f32, tag="bf1")
                nc.sync.dma_start(
                    out=bf1_sb, in_=opt["bf1"][:].rearrange("(t p) -> p t", p=P)
                )

            xp_r = xp_e.rearrange("(t p) c -> p t c", p=P)
            xk_r = xk_e.rearrange("(t p) c -> p t c", p=P)

            mask_v = flags["ln1_gb"] or flags["bv"]

            for _rep in range(repeat):
                _mark(nc, "setup")
                # own x rows (residual + LN1 input), compacted key rows
                xt_own = big.tile([P, NT_O, DIM], f32, tag="xt_own")
                for t0 in range(0, NT_O, 2):
                    nc.sync.dma_start(
                        out=xt_own[:, t0 : t0 + 2, :], in_=xp_r[:, t0 : t0 + 2, :]
                    )
                xk_sb = big.tile([P, NKT, DIM], bf16, tag="xk")
                nc.sync.dma_start(out=xk_sb, in_=xk_r)

                wqk_sb = big.tile([P, KC, 2 * DIM], fp8, tag="wqk")
                nc.sync.dma_start(
                    out=wqk_sb, in_=wqk_e.rearrange("(k p) c -> p k c", p=P)
                )
                wv_sb = big.tile([P, KC, DIM], fp8, tag="wv")
                nc.sync.dma_start(
                    out=wv_sb, in_=wv_e.rearrange("(k p) c -> p k c", p=P)
                )

                # --- LN1 + xbar transpose + fp8 cast: hT (own) / hkT (keys) ---
                _mark(nc, "ln1_hT")
                hT = big.tile([P, KC, 512], fp8, tag="hT")
                hkT = big.tile([P, KC, NK], fp8, tag="hkT")
                for grp in (range(NT_O), range(NT_O, NT_O + NKT)):
                    grp = list(grp)
                    mv_g = lnp.tile([P, len(grp), 2], f32, tag="ln_mv")
                    for i, t in enumerate(grp):
                        own = t < NT_O
                        x_ap = xt_own[:, t, :] if own else xk_sb[:, t - NT_O, :]
                        _ln_stats(nc, lnp, x_ap, mv_g, i)
                    rstd_g = _ln_rstd_group(nc, lnp, mv_g, eps_t, len(grp))
                    for i, t in enumerate(grp):
                        own = t < NT_O
                        x_ap = xt_own[:, t, :] if own else xk_sb[:, t - NT_O, :]
                        h_t = htmp.tile([P, DIM], bf16, tag="h")
                        _ln_apply(
                            nc, x_ap, h_t, mv_g, rstd_g, i,
                            reps.get("ln1g"), reps.get("ln1b"),
                        )
                        tt = ttmp.tile([P, KC, P], bf16, tag="tt")
                        nc.scalar.dma_start_transpose(out=tt, in_=h_t)
                        if own:
                            dst = hT[:, :, t * P : (t + 1) * P]
                        else:
                            tk = t - NT_O
                            dst = hkT[:, :, tk * P : (tk + 1) * P]
                        nc.gpsimd.tensor_copy(out=dst, in_=tt)

                # --- qT (bf16, own tokens) ---
                _mark(nc, "qkT")
                qT = big.tile([P, KC, 512], bf16, tag="qT")
                pairs, tail = _dr_steps(KC)
                for mt in range(KC):
                    ps = mmps.tile([P, 512], f32, tag="mm", name="mm")
                    for j in range(pairs):
                        nc.tensor.matmul(
                            ps,
                            lhsT=wqk_sb[:, 2 * j : 2 * j + 2, mt * P : (mt + 1) * P],
                            rhs=hT[:, 2 * j : 2 * j + 2, :],
                            start=(j == 0), stop=(j == pairs - 1 and not tail),
                            perf_mode=DR,
                        )
                    if bqk_sb is not None:
                        nc.vector.tensor_scalar_add(
                            out=qT[:, mt, :], in0=ps, scalar1=bqk_sb[:, mt : mt + 1]
                        )
                    else:
                        nc.vector.tensor_copy(out=qT[:, mt, :], in_=ps)

                # --- kT (bf16, compacted keys) ---
                kT = big.tile([P, KC, NK], bf16, tag="kT")
                for mt in range(KC):
                    for n0 in range(0, NK, 512):
                        n1 = min(n0 + 512, NK)
                        ps_f = mmps.tile([P, 512], f32, tag="mm", name="mm")
                        ps = ps_f[:, : n1 - n0]
                        for j in range(pairs):
                            nc.tensor.matmul(
                                ps,
                                lhsT=wqk_sb[
                                    :, 2 * j : 2 * j + 2,
                                    DIM + mt * P : DIM + (mt + 1) * P,
                                ],
                                rhs=hkT[:, 2 * j : 2 * j + 2, n0:n1],
                                start=(j == 0), stop=(j == pairs - 1 and not tail),
                                perf_mode=DR,
                            )
                        kbias = (
                            bqk_sb[:, KC + mt : KC + mt + 1]
                            if bqk_sb is not None else 0.0
                        )
                        nc.scalar.activation(
                            out=kT[:, mt, n0:n1], in_=ps, func=ACT_F.Copy,
                            bias=kbias, scale=1.0,
                        )

                # --- v (natural rows over keys), packed per head + ones col ---
                _mark(nc, "v")
                v_aug = big.tile([P, NKT, HEADS * 65], fp8, tag="vaug")
                v_aug_h = v_aug.rearrange("p t (h c) -> p t h c", c=65)
                m01_bc = bass.AP(
                    tensor=m01k_sb.tensor, offset=m01k_sb.offset,
                    ap=[m01k_sb.ap[0], m01k_sb.ap[1], [0, HEADS], [0, 1]],
                )
                nc.gpsimd.tensor_copy(out=v_aug_h[:, :, :, 64:65], in_=m01_bc)
                for t in range(NKT):
                    for n0, n1 in ((0, 512), (512, 768)):
                        ps_f = mmps.tile([P, 512], f32, tag="mm", name="mm")
                        ps = ps_f[:, : n1 - n0]
                        for j in range(pairs):
                            nc.tensor.matmul(
                                ps,
                                lhsT=hkT[:, 2 * j : 2 * j + 2, t * P : (t + 1) * P],
                                rhs=wv_sb[:, 2 * j : 2 * j + 2, n0:n1],
                                start=(j == 0), stop=(j == pairs - 1),
                                perf_mode=DR,
                            )
                        h0, h1 = n0 // HD, n1 // HD
                        dst = v_aug_h[:, t, h0:h1, 0:HD]
                        src = ps.rearrange("p (h c) -> p h c", c=HD)
                        if "bv" in reps:
                            nc.vector.tensor_add(
                                out=dst, in0=src,
                                in1=reps["bv"][:, n0:n1].rearrange(
                                    "p (h c) -> p h c", c=HD
                                ),
                            )
                        else:
                            nc.vector.tensor_copy(out=dst, in_=src)
                        if mask_v:
                            nc.gpsimd.tensor_scalar_mul(
                                out=dst, in0=dst, scalar1=m01k_sb[:, t : t + 1]
                            )

                wp_sb = big.tile([P, KC, DIM], fp8, tag="wp")
                nc.sync.dma_start(
                    out=wp_sb, in_=wp_e.rearrange("(k p) c -> p k c", p=P)
                )
                wf1_sb = big.tile([P, KC, HID], fp8 if fc1_fp8 else bf16, tag="wf1")
                nc.sync.dma_start(
                    out=wf1_sb, in_=wf1_e.rearrange("(k p) c -> p k c", p=P)
                )
                wf2_sb = big.tile([P, KH, DIM], fp8 if fc2_fp8 else bf16, tag="wf2")
                for k0 in range(0, KH, 8):
                    nc.sync.dma_start(
                        out=wf2_sb[:, k0 : k0 + 8, :],
                        in_=wf2_e.rearrange("(k p) c -> p k c", p=P)[:, k0 : k0 + 8, :],
                    )

                # --- attention: scores (bf16) -> exp (Act) -> av (fp8 DR) ---
                _mark(nc, "attn")
                esc = float(HD) ** -0.5 / (SW * SW)
                o_sb = big.tile([P, NT_O, DIM], bf16, tag="o")
                kp, ktail = _dr_steps(NKT)
                for hp in range(HEADS // 2):
                    pT = ppool.tile([P, NKT, 2, 512], fp8, tag="pT")
                    for m in range(NKT):
                        ps = sps.tile([P, 2, 512], f32, tag="s")
                        for sub in range(2):
                            base = sub * HD
                            nc.tensor.matmul(
                                ps[:, sub, :],
                                lhsT=kT[base : base + HD, hp, m * P : (m + 1) * P],
                                rhs=qT[base : base + HD, hp, :],
                                start=True, stop=True,
                            )
                        nc.scalar.activation(
                            out=pT[:, m, :, :], in_=ps, func=ACT_F.Exp, scale=esc
                        )
                    for sub in range(2):
                        h = 2 * hp + sub
                        for nt in range(NT_O):
                            po_f = mmps.tile([P, 512], f32, tag="mm", name="mm")
                            po = po_f[:, :65]
                            for i in range(kp):
                                nc.tensor.matmul(
                                    po,
                                    lhsT=pT[
                                        :, 2 * i : 2 * i + 2, sub,
                                        nt * P : (nt + 1) * P,
                                    ],
                                    rhs=v_aug_h[:, 2 * i : 2 * i + 2, h, :],
                                    start=(i == 0), stop=(i == kp - 1 and not ktail),
                                    perf_mode=DR,
                                )
                            if ktail:
                                nc.tensor.matmul(
                                    po,
                                    lhsT=pT[:, NKT - 1, sub, nt * P : (nt + 1) * P],
                                    rhs=v_aug_h[:, NKT - 1, h, :],
                                    start=(kp == 0), stop=True,
                                )
                            rcp = lnp.tile([P, 1], f32, tag="rcp")
                            nc.vector.reciprocal(out=rcp, in_=po[:, 64:65])
                            nc.vector.tensor_scalar_mul(
                                out=o_sb[:, nt, h * HD : (h + 1) * HD],
                                in0=po[:, 0:HD], scalar1=rcp,
                            )

                # --- oT via xbar + cast ---
                _mark(nc, "oT")
                oT = big.tile([P, KC, 512], fp8, tag="hT")
                for nt in range(NT_O):
                    tt = ttmp.tile([P, KC, P], bf16, tag="tt")
                    nc.scalar.dma_start_transpose(out=tt, in_=o_sb[:, nt, :])
                    nc.gpsimd.tensor_copy(
                        out=oT[:, :, nt * P : (nt + 1) * P], in_=tt
                    )

                # --- proj (fp8 DR) + unscale + residual -> xmid ---
                _mark(nc, "proj")
                xmid = big.tile([P, NT_O, DIM], f32, tag="xmid")
                for nt in range(NT_O):
                    for n0, n1 in ((0, 512), (512, 768)):
                        ps_f = mmps.tile([P, 512], f32, tag="mm", name="mm")
                        ps = ps_f[:, : n1 - n0]
                        for j in range(pairs):
                            nc.tensor.matmul(
                                ps,
                                lhsT=oT[:, 2 * j : 2 * j + 2, nt * P : (nt + 1) * P],
                                rhs=wp_sb[:, 2 * j : 2 * j + 2, n0:n1],
                                start=(j == 0), stop=(j == pairs - 1),
                                perf_mode=DR,
                            )
                        nc.vector.scalar_tensor_tensor(
                            out=xmid[:, nt, n0:n1], in0=ps,
                            scalar=1.0 / (SW * SW), in1=xt_own[:, nt, n0:n1],
                            op0=ALU.mult, op1=ALU.add,
                        )
                        if "bp" in reps:
                            nc.gpsimd.tensor_add(
                                out=xmid[:, nt, n0:n1], in0=xmid[:, nt, n0:n1],
                                in1=reps["bp"][:, n0:n1],
                            )

                # --- LN2 + xbar transpose (+ cast if fc1 fp8) -> h2T ---
                _mark(nc, "ln2")
                h2T = big.tile([P, KC, 512], fp8 if fc1_fp8 else bf16, tag="h2T")
                mv2_g = lnp.tile([P, NT_O, 2], f32, tag="ln_mv")
                for nt in range(NT_O):
                    _ln_stats(nc, lnp, xmid[:, nt, :], mv2_g, nt)
                rstd2_g = _ln_rstd_group(nc, lnp, mv2_g, eps_t, NT_O)
                for nt in range(NT_O):
                    h_t = htmp.tile([P, DIM], bf16, tag="h")
                    _ln_apply(
                        nc, xmid[:, nt, :], h_t, mv2_g, rstd2_g, nt,
                        reps.get("ln2g"), reps.get("ln2b"),
                    )
                    if fc1_fp8:
                        tt = ttmp.tile([P, KC, P], bf16, tag="tt")
                        nc.scalar.dma_start_transpose(out=tt, in_=h_t)
                        nc.gpsimd.tensor_copy(
                            out=h2T[:, :, nt * P : (nt + 1) * P], in_=tt
                        )
                    else:
                        nc.scalar.dma_start_transpose(
                            out=h2T[:, :, nt * P : (nt + 1) * P], in_=h_t
                        )

                # --- fc1 + gelu -> g1T ---
                _mark(nc, "fc1")
                g1T = big.tile([P, KH, 512], fp8 if fc2_fp8 else bf16, tag="g1T")
                gsc = 1.0 / SW if fc1_fp8 else 1.0
                for mt in range(KH):
                    ps = mmps.tile([P, 512], f32, tag="mm", name="mm")
                    if fc1_fp8:
                        for j in range(pairs):
                            nc.tensor.matmul(
                                ps,
                                lhsT=wf1_sb[
                                    :, 2 * j : 2 * j + 2, mt * P : (mt + 1) * P
                                ],
                                rhs=h2T[:, 2 * j : 2 * j + 2, :],
                                start=(j == 0), stop=(j == pairs - 1),
                                perf_mode=DR,
                            )
                    else:
                        for k in range(KC):
                            nc.tensor.matmul(
                                ps,
                                lhsT=wf1_sb[:, k, mt * P : (mt + 1) * P],
                                rhs=h2T[:, k, :],
                                start=(k == 0), stop=(k == KC - 1),
                            )
                    gl_bias = bf1_sb[:, mt : mt + 1] if bf1_sb is not None else 0.0
                    nc.scalar.activation(
                        out=g1T[:, mt, :], in_=ps, func=ACT_F.Gelu,
                        bias=gl_bias, scale=gsc,
                    )

                # --- fc2 + residual -> y ---
                _mark(nc, "fc2")
                y_sb = big.tile([P, NT_O, DIM], f32, tag="xt_own")
                y_r = y_e.rearrange("(t p) c -> p t c", p=P)
                kp2, ktail2 = _dr_steps(KH)
                for nt in range(NT_O):
                    for n0, n1 in ((0, 512), (512, 768)):
                        ps_f = mmps.tile([P, 512], f32, tag="mm", name="mm")
                        ps = ps_f[:, : n1 - n0]
                        if fc2_fp8:
                            for i in range(kp2):
                                nc.tensor.matmul(
                                    ps,
                                    lhsT=g1T[
                                        :, 2 * i : 2 * i + 2, nt * P : (nt + 1) * P
                                    ],
                                    rhs=wf2_sb[:, 2 * i : 2 * i + 2, n0:n1],
                                    start=(i == 0), stop=(i == kp2 - 1),
                                    perf_mode=DR,
                                )
                            nc.vector.scalar_tensor_tensor(
                                out=y_sb[:, nt, n0:n1], in0=ps, scalar=1.0 / SW,
                                in1=xmid[:, nt, n0:n1], op0=ALU.mult, op1=ALU.add,
                            )
                        else:
                            for k in range(KH):
                                nc.tensor.matmul(
                                    ps,
                                    lhsT=g1T[:, k, nt * P : (nt + 1) * P],
                                    rhs=wf2_sb[:, k, n0:n1],
                                    start=(k == 0), stop=(k == KH - 1),
                                )
                            nc.vector.tensor_add(
                                out=y_sb[:, nt, n0:n1], in0=ps,
                                in1=xmid[:, nt, n0:n1],
                            )
                        if "bf2" in reps:
                            nc.gpsimd.tensor_add(
                                out=y_sb[:, nt, n0:n1], in0=y_sb[:, nt, n0:n1],
                                in1=reps["bf2"][:, n0:n1],
                            )
                        nc.sync.dma_start(
                            out=y_r[:, nt, n0:n1], in_=y_sb[:, nt, n0:n1]
                        )

    nc.finalize()
    return nc


def _nontriv(a, val):
    return not np.allclose(np.asarray(a), val, rtol=0, atol=0)


_last_flags = None


def _prepare(x, attention_mask, ln1_g, ln1_b, ln2_g, ln2_b,
             w_qkv, b_qkv, w_proj, b_proj, w_fc1, b_fc1, w_fc2, b_fc2):
    x = np.ascontiguousarray(np.asarray(x, np.float32))
    attention_mask = np.asarray(attention_mask)
    B, N, C = x.shape
    H = N // 2  # 512

    kept = [np.nonzero(attention_mask[b] != 0)[0] for b in range(B)]
    maxk = max(len(k) for k in kept)
    assert maxk > 0, "all keys masked is not supported"
    NKT = -(-maxk // P)
    NK = NKT * P

    flags = {
        "ln1_gb": _nontriv(ln1_g, 1.0) or _nontriv(ln1_b, 0.0),
        "ln2_gb": _nontriv(ln2_g, 1.0) or _nontriv(ln2_b, 0.0),
        "bqk": _nontriv(b_qkv[: 2 * DIM], 0.0),
        "bv": _nontriv(b_qkv[2 * DIM :], 0.0),
        "bp": _nontriv(b_proj, 0.0),
        "bf1": _nontriv(b_fc1, 0.0),
        "bf2": _nontriv(b_fc2, 0.0),
        "nkt": NKT,
        "fc1_fp8": False,
        "fc2_fp8": True,
    }

    e4 = ml_dtypes.float8_e4m3fn
    w_qkv = np.asarray(w_qkv, np.float32)
    wqk = np.ascontiguousarray(w_qkv[:, : 2 * DIM] * SW).astype(e4)
    wv = np.ascontiguousarray(w_qkv[:, 2 * DIM :] * SW).astype(e4)
    wp = (np.asarray(w_proj, np.float32) * SW).astype(e4)
    if flags["fc1_fp8"]:
        wf1 = (np.asarray(w_fc1, np.float32) * SW).astype(e4)
    else:
        wf1 = np.asarray(w_fc1, np.float32).astype(ml_dtypes.bfloat16)
    if flags["fc2_fp8"]:
        wf2 = (np.asarray(w_fc2, np.float32) * SW).astype(e4)
    else:
        wf2 = np.asarray(w_fc2, np.float32).astype(ml_dtypes.bfloat16)

    shared = {"wqk": wqk, "wv": wv, "wp": wp, "wf1": wf1, "wf2": wf2}
    if flags["ln1_gb"]:
        shared["ln1g"] = np.asarray(ln1_g, np.float32)
        shared["ln1b"] = np.asarray(ln1_b, np.float32)
    if flags["ln2_gb"]:
        shared["ln2g"] = np.asarray(ln2_g, np.float32)
        shared["ln2b"] = np.asarray(ln2_b, np.float32)
    if flags["bqk"]:
        shared["bqk"] = np.asarray(b_qkv[: 2 * DIM], np.float32) * SW
    if flags["bv"]:
        shared["bv"] = np.asarray(b_qkv[2 * DIM :], np.float32) * SW
    if flags["bp"]:
        shared["bp"] = np.asarray(b_proj, np.float32)
    if flags["bf1"]:
        shared["bf1"] = np.asarray(b_fc1, np.float32)
    if flags["bf2"]:
        shared["bf2"] = np.asarray(b_fc2, np.float32)

    per_batch = []
    for b in range(B):
        idx = kept[b]
        xkb = np.zeros((NK, C), np.float32)
        xkb[: len(idx)] = x[b, idx]
        m = np.zeros((NK,), np.float32)
        m[: len(idx)] = 1.0
        per_batch.append(
            (
                np.ascontiguousarray(xkb).astype(ml_dtypes.bfloat16),
                np.ascontiguousarray(m.reshape(NKT, P).T),
            )
        )

    in_maps = []
    for c in range(N_CORES):
        b, hf = divmod(c, 2)
        xk_b, m01k_b = per_batch[b]
        in_maps.append(
            {
                "xp": np.ascontiguousarray(x[b, hf * H : (hf + 1) * H]),
                "xk": xk_b,
                "m01k": m01k_b,
                **shared,
            }
        )

    global _last_flags
    _last_flags = flags
    nc = _build(flags)
    return nc, in_maps, (B, N, C)


def kernel(**inputs):
    nc, in_maps, (B, N, C) = _prepare(**inputs)
    res = run_bass_kernel_spmd(nc, in_maps, list(range(N_CORES)))
    out = np.empty((B, N, C), np.float32)
    H = N // 2
    for c in range(N_CORES):
        b, hf = divmod(c, 2)
        out[b, hf * H : (hf + 1) * H] = res.results[c]["y"]
    return out


# revision 34
# speedup vs baseline: 7.5264x; 7.5264x over previous
"""Fused transformer block (B=4, N=1024, C=768, H=12, HID=3072) on 8 TRN2
NeuronCores.

Sharding: data-parallel over (batch, seq-half): core c handles batch c//2,
sequence half c%2 -> 512 query rows. Keys/values are computed per core from a
host-side COMPACTED key list: the attention mask zeroes ~half the keys and
their softmax weight in the reference is exactly 0 (exp(-10000) underflows),
so masked keys are dropped on the host and the key set padded to a multiple
of 128 (pad rows are zero, their v/denominator columns are masked to 0).

The kernel is PE-sequencer-issue-bound, so the design minimizes matmul
instruction count:
  - fp8(e4m3) DoubleRow matmuls (contraction 256/instruction) for qkv, av,
    proj and fc1; weights are scaled x32 on the host into fp8's sweet spot
    and the 1/32 factors folded into activation scales / epilogues.
  - transposes run on the DMA xbar (bf16), not the PE.
  - scores and fc2 stay bf16 for accuracy; softmax normalizer comes from an
    extra ones-column in v (col 64 of each head block), divided out on DVE.
  - LN rstd = exp(-0.5*ln(var+eps)) so the Act engine only ever needs the
    natural_log_exp table plus one swap to the gelu table for fc1.
Vector work is split DVE (bn_stats, divides, psum->bf16 copies) vs Pool
(LN apply, fp8 casts, v_aug packing, residual epilogues).
"""

import numpy as np
import ml_dtypes

import concourse.bass as bass
import concourse.bacc as bacc
import concourse.mybir as mybir
import concourse.tile as tile
from concourse.bass_utils import run_bass_kernel_spmd

P = 128
DIM = 768
HEADS = 12
HD = 64
HID = 3072
EPS = 1e-5
NT_O = 4  # token tiles for the core's own 512 rows
KC = DIM // P  # 6
KH = HID // P  # 24
N_CORES = 8
SW = 32.0  # host-side fp8 weight scale

bf16 = mybir.dt.bfloat16
fp8 = mybir.dt.float8e4
f32 = mybir.dt.float32
ALU = mybir.AluOpType
ACT_F = mybir.ActivationFunctionType
DR = mybir.MatmulPerfMode.DoubleRow

_PHASE_MARKS = []  # (phase_name, first_instruction_number); for sim analysis


def _mark(nc, phase):
    _PHASE_MARKS.append((phase, int(nc.get_next_instruction_name().split("-")[1])))


def _ln_stats(nc, lnp, x_ap, mv_g, slot):
    """bn stats for one [128,768] tile -> mean/var into mv_g[:, slot, :]."""
    stats = lnp.tile([P, 3, 6], f32, tag="ln_stats")
    xg = x_ap.rearrange("p (s d) -> p s d", s=3)
    for s in range(3):
        nc.vector.bn_stats(out=stats[:, s, :], in_=xg[:, s, :])
    nc.vector.bn_aggr(out=mv_g[:, slot, :], in_=stats)


def _ln_stats_act(nc, lnp, scr, x_ap, mv_g, slot):
    """Same as _ln_stats but on the Act engine via free-dim accumulators
    (Copy -> sum(x), Square -> sum(x^2); both funcs are in every act table).
    Frees the DVE on the LN critical path; 3 tiny DVE ops derive mean/var."""
    junk = scr.tile([P, DIM], bf16, tag="ln_junk")
    sx = lnp.tile([P, 1], f32, tag="ln_sx")
    nc.scalar.activation(out=junk, in_=x_ap, func=ACT_F.Copy, accum_out=sx)
    sxx = lnp.tile([P, 1], f32, tag="ln_sxx")
    nc.scalar.activation(out=junk, in_=x_ap, func=ACT_F.Square, accum_out=sxx)
    nc.vector.tensor_scalar_mul(
        out=mv_g[:, slot, 0:1], in0=sx, scalar1=1.0 / DIM
    )
    m2 = lnp.tile([P, 1], f32, tag="ln_m2")
    nc.vector.tensor_mul(
        out=m2, in0=mv_g[:, slot, 0:1], in1=mv_g[:, slot, 0:1]
    )
    nc.vector.scalar_tensor_tensor(
        out=mv_g[:, slot, 1:2], in0=sxx, scalar=1.0 / DIM, in1=m2,
        op0=ALU.mult, op1=ALU.subtract,
    )


def _ln_rstd_group(nc, lnp, mv_g, eps_t, n):
    """rstd for a group of n LN tiles in 2 Act ops: exp(-0.5*ln(var+eps)).
    Both funcs live in the natural_log_exp table (same one the softmax exp
    uses), so no activation-table swaps are ever needed outside gelu."""
    lnv = lnp.tile([P, n], f32, tag="ln_lnv")
    nc.scalar.activation(
        out=lnv, in_=mv_g[:, 0:n, 1], func=ACT_F.Ln, bias=eps_t, scale=1.0
    )
    rstd_g = lnp.tile([P, n], f32, tag="ln_rstd")
    nc.scalar.activation(out=rstd_g, in_=lnv, func=ACT_F.Exp, bias=0.0, scale=-0.5)
    return rstd_g


def _ln_apply(nc, x_ap, out_ap, mv_g, rstd_g, slot, g_rep, b_rep):
    nc.vector.tensor_scalar(
        out=out_ap, in0=x_ap, scalar1=mv_g[:, slot, 0:1],
        scalar2=rstd_g[:, slot : slot + 1], op0=ALU.subtract, op1=ALU.mult,
    )
    if g_rep is not None:
        nc.gpsimd.tensor_mul(out=out_ap, in0=out_ap, in1=g_rep)
    if b_rep is not None:
        nc.gpsimd.tensor_add(out=out_ap, in0=out_ap, in1=b_rep)


def _dr_steps(nkt):
    """(pairs, has_tail) for a DoubleRow contraction over nkt 128-tiles."""
    return nkt // 2, nkt % 2


def _build(flags, repeat=1):
    nc = bacc.Bacc(None)
    _PHASE_MARKS.clear()
    NKT = flags["nkt"]
    NK = NKT * P
    fc1_fp8 = flags["fc1_fp8"]
    fc2_fp8 = flags["fc2_fp8"]

    xp_e = nc.declare_dram_parameter("xp", [512, DIM], f32, isOutput=False)
    xk_e = nc.declare_dram_parameter("xk", [NK, DIM], bf16, isOutput=False)
    m01k_e = nc.declare_dram_parameter("m01k", [P, NKT], f32, isOutput=False)
    wqk_e = nc.declare_dram_parameter("wqk", [DIM, 2 * DIM], fp8, isOutput=False)
    wv_e = nc.declare_dram_parameter("wv", [DIM, DIM], fp8, isOutput=False)
    wp_e = nc.declare_dram_parameter("wp", [DIM, DIM], fp8, isOutput=False)
    wf1_e = nc.declare_dram_parameter(
        "wf1", [DIM, HID], fp8 if fc1_fp8 else bf16, isOutput=False
    )
    wf2_e = nc.declare_dram_parameter(
        "wf2", [HID, DIM], fp8 if fc2_fp8 else bf16, isOutput=False
    )
    y_e = nc.declare_dram_parameter("y", [512, DIM], f32, isOutput=True)

    opt = {}
    for name, dim, fl in (
        ("ln1g", DIM, "ln1_gb"), ("ln1b", DIM, "ln1_gb"),
        ("ln2g", DIM, "ln2_gb"), ("ln2b", DIM, "ln2_gb"),
        ("bqk", 2 * DIM, "bqk"), ("bv", DIM, "bv"), ("bp", DIM, "bp"),
        ("bf1", HID, "bf1"), ("bf2", DIM, "bf2"),
    ):
        if flags[fl]:
            opt[name] = nc.declare_dram_parameter(name, [dim], f32, isOutput=False)

    def bcast(ap):
        # replicate a [D] DRAM vector across all 128 partitions for DMA
        return bass.AP(tensor=ap.tensor, offset=ap.offset, ap=[[0, P], *ap.ap])

    with tile.TileContext(nc) as tc:
        import contextlib

        with contextlib.ExitStack() as ctx:
            singles = ctx.enter_context(tc.tile_pool(name="singles", bufs=1))
            lnp = ctx.enter_context(tc.tile_pool(name="ln", bufs=4))
            htmp = ctx.enter_context(tc.tile_pool(name="htmp", bufs=3))
            ttmp = ctx.enter_context(tc.tile_pool(name="ttmp", bufs=3))
            scr = ctx.enter_context(tc.tile_pool(name="scr", bufs=2))
            big = ctx.enter_context(tc.tile_pool(name="big", bufs=1))
            ppool = ctx.enter_context(tc.tile_pool(name="pT", bufs=3))
            mmps = ctx.enter_context(tc.tile_pool(name="mmps", bufs=4, space="PSUM"))
            sps = ctx.enter_context(tc.tile_pool(name="sps", bufs=2, space="PSUM"))

            # --- constants ---
            eps_t = singles.tile([P, 1], f32)
            nc.vector.memset(eps_t, EPS)
            m01k_sb = singles.tile([P, NKT], f32)
            nc.sync.dma_start(out=m01k_sb, in_=m01k_e[:, :])

            reps = {}
            for name in ("ln1g", "ln1b", "ln2g", "ln2b", "bv", "bp", "bf2"):
                if name in opt:
                    r = singles.tile([P, opt[name].shape[0]], f32, tag=name)
                    nc.sync.dma_start(out=r, in_=bcast(opt[name][:]))
                    reps[name] = r
            bqk_sb = bf1_sb = None
            if "bqk" in opt:
                bqk_sb = singles.tile([P, 2 * KC], f32, tag="bqk")
                nc.sync.dma_start(
                    out=bqk_sb, in_=opt["bqk"][:].rearrange("(t p) -> p t", p=P)
                )
            if "bf1" in opt:
                bf1_sb = singles.tile([P, KH], f32, tag="bf1")
                nc.sync.dma_start(
                    out=bf1_sb, in_=opt["bf1"][:].rearrange("(t p) -> p t", p=P)
                )

            xp_r = xp_e.rearrange("(t p) c -> p t c", p=P)
            xk_r = xk_e.rearrange("(t p) c -> p t c", p=P)

            mask_v = flags["ln1_gb"] or flags["bv"]

            for _rep in range(repeat):
                _mark(nc, "setup")
                # own x rows (residual + LN1 input), compacted key rows
                xt_own = big.tile([P, NT_O, DIM], f32, tag="xt_own")
                for t0 in range(0, NT_O, 2):
                    nc.sync.dma_start(
                        out=xt_own[:, t0 : t0 + 2, :], in_=xp_r[:, t0 : t0 + 2, :]
                    )
                xk_sb = big.tile([P, NKT, DIM], bf16, tag="xk")
                nc.sync.dma_start(out=xk_sb, in_=xk_r)

                wqk_sb = big.tile([P, KC, 2 * DIM], fp8, tag="wqk")
                nc.sync.dma_start(
                    out=wqk_sb, in_=wqk_e.rearrange("(k p) c -> p k c", p=P)
                )
                wv_sb = big.tile([P, KC, DIM], fp8, tag="wv")
                nc.sync.dma_start(
                    out=wv_sb, in_=wv_e.rearrange("(k p) c -> p k c", p=P)
                )

                # --- LN1 + xbar transpose + fp8 cast: hT (own) / hkT (keys) ---
                _mark(nc, "ln1_hT")
                hT = big.tile([P, KC, 512], fp8, tag="hT")
                hkT = big.tile([P, KC, NK], fp8, tag="hkT")
                for grp in (range(NT_O), range(NT_O, NT_O + NKT)):
                    grp = list(grp)
                    mv_g = lnp.tile([P, len(grp), 2], f32, tag="ln_mv")
                    for i, t in enumerate(grp):
                        own = t < NT_O
                        x_ap = xt_own[:, t, :] if own else xk_sb[:, t - NT_O, :]
                        if own:
                            _ln_stats_act(nc, lnp, scr, x_ap, mv_g, i)
                        else:
                            _ln_stats(nc, lnp, x_ap, mv_g, i)
                    rstd_g = _ln_rstd_group(nc, lnp, mv_g, eps_t, len(grp))
                    for i, t in enumerate(grp):
                        own = t < NT_O
                        x_ap = xt_own[:, t, :] if own else xk_sb[:, t - NT_O, :]
                        h_t = htmp.tile([P, DIM], bf16, tag="h")
                        _ln_apply(
                            nc, x_ap, h_t, mv_g, rstd_g, i,
                            reps.get("ln1g"), reps.get("ln1b"),
                        )
                        tt = ttmp.tile([P, KC, P], bf16, tag="tt")
                        nc.scalar.dma_start_transpose(out=tt, in_=h_t)
                        if own:
                            dst = hT[:, :, t * P : (t + 1) * P]
                        else:
                            tk = t - NT_O
                            dst = hkT[:, :, tk * P : (tk + 1) * P]
                        nc.gpsimd.tensor_copy(out=dst, in_=tt)

                # --- qT (bf16, own tokens) ---
                _mark(nc, "qkT")
                qT = big.tile([P, KC, 512], bf16, tag="qT")
                pairs, tail = _dr_steps(KC)
                for mt in range(KC):
                    ps = mmps.tile([P, 512], f32, tag="mm", name="mm")
                    for j in range(pairs):
                        nc.tensor.matmul(
                            ps,
                            lhsT=wqk_sb[:, 2 * j : 2 * j + 2, mt * P : (mt + 1) * P],
                            rhs=hT[:, 2 * j : 2 * j + 2, :],
                            start=(j == 0), stop=(j == pairs - 1 and not tail),
                            perf_mode=DR,
                        )
                    if bqk_sb is not None:
                        nc.vector.tensor_scalar_add(
                            out=qT[:, mt, :], in0=ps, scalar1=bqk_sb[:, mt : mt + 1]
                        )
                    else:
                        nc.vector.tensor_copy(out=qT[:, mt, :], in_=ps)

                # --- kT (bf16, compacted keys) ---
                kT = big.tile([P, KC, NK], bf16, tag="kT")
                for mt in range(KC):
                    for n0 in range(0, NK, 512):
                        n1 = min(n0 + 512, NK)
                        ps_f = mmps.tile([P, 512], f32, tag="mm", name="mm")
                        ps = ps_f[:, : n1 - n0]
                        for j in range(pairs):
                            nc.tensor.matmul(
                                ps,
                                lhsT=wqk_sb[
                                    :, 2 * j : 2 * j + 2,
                                    DIM + mt * P : DIM + (mt + 1) * P,
                                ],
                                rhs=hkT[:, 2 * j : 2 * j + 2, n0:n1],
                                start=(j == 0), stop=(j == pairs - 1 and not tail),
                                perf_mode=DR,
                            )
                        kbias = (
                            bqk_sb[:, KC + mt : KC + mt + 1]
                            if bqk_sb is not None else 0.0
                        )
                        nc.scalar.activation(
                            out=kT[:, mt, n0:n1], in_=ps, func=ACT_F.Copy,
                            bias=kbias, scale=1.0,
                        )

                # --- v (natural rows over keys), packed per head + ones col ---
                _mark(nc, "v")
                v_aug = big.tile([P, NKT, HEADS * 65], fp8, tag="vaug")
                v_aug_h = v_aug.rearrange("p t (h c) -> p t h c", c=65)
                m01_bc = bass.AP(
                    tensor=m01k_sb.tensor, offset=m01k_sb.offset,
                    ap=[m01k_sb.ap[0], m01k_sb.ap[1], [0, HEADS], [0, 1]],
                )
                nc.gpsimd.tensor_copy(out=v_aug_h[:, :, :, 64:65], in_=m01_bc)
                for t in range(NKT):
                    for n0, n1 in ((0, 512), (512, 768)):
                        ps_f = mmps.tile([P, 512], f32, tag="mm", name="mm")
                        ps = ps_f[:, : n1 - n0]
                        for j in range(pairs):
                            nc.tensor.matmul(
                                ps,
                                lhsT=hkT[:, 2 * j : 2 * j + 2, t * P : (t + 1) * P],
                                rhs=wv_sb[:, 2 * j : 2 * j + 2, n0:n1],
                                start=(j == 0), stop=(j == pairs - 1),
                                perf_mode=DR,
                            )
                        h0, h1 = n0 // HD, n1 // HD
                        dst = v_aug_h[:, t, h0:h1, 0:HD]
                        src = ps.rearrange("p (h c) -> p h c", c=HD)
                        if "bv" in reps:
                            nc.vector.tensor_add(
                                out=dst, in0=src,
                                in1=reps["bv"][:, n0:n1].rearrange(
                                    "p (h c) -> p h c", c=HD
                                ),
                            )
                        else:
                            nc.vector.tensor_copy(out=dst, in_=src)
                        if mask_v:
                            nc.gpsimd.tensor_scalar_mul(
                                out=dst, in0=dst, scalar1=m01k_sb[:, t : t + 1]
                            )

                wp_sb = big.tile([P, KC, DIM], fp8, tag="wp")
                nc.sync.dma_start(
                    out=wp_sb, in_=wp_e.rearrange("(k p) c -> p k c", p=P)
                )
                wf1_sb = big.tile([P, KC, HID], fp8 if fc1_fp8 else bf16, tag="wf1")
                for k0 in range(0, KC, 2):
                    nc.sync.dma_start(
                        out=wf1_sb[:, k0 : k0 + 2, :],
                        in_=wf1_e.rearrange("(k p) c -> p k c", p=P)[:, k0 : k0 + 2, :],
                    )
                wf2_sb = big.tile([P, KH, DIM], fp8 if fc2_fp8 else bf16, tag="wf2")
                for k0 in range(0, KH, 8):
                    nc.sync.dma_start(
                        out=wf2_sb[:, k0 : k0 + 8, :],
                        in_=wf2_e.rearrange("(k p) c -> p k c", p=P)[:, k0 : k0 + 8, :],
                    )

                # --- attention: scores (bf16) -> exp (Act) -> av (fp8 DR) ---
                _mark(nc, "attn")
                esc = float(HD) ** -0.5 / (SW * SW)
                o_sb = big.tile([P, NT_O, DIM], bf16, tag="o")
                kp, ktail = _dr_steps(NKT)
                for hp in range(HEADS // 2):
                    pT = ppool.tile([P, NKT, 2, 512], fp8, tag="pT")
                    for m in range(NKT):
                        ps = sps.tile([P, 2, 512], f32, tag="s")
                        for sub in range(2):
                            base = sub * HD
                            nc.tensor.matmul(
                                ps[:, sub, :],
                                lhsT=kT[base : base + HD, hp, m * P : (m + 1) * P],
                                rhs=qT[base : base + HD, hp, :],
                                start=True, stop=True,
                            )
                        nc.scalar.activation(
                            out=pT[:, m, :, :], in_=ps, func=ACT_F.Exp, scale=esc
                        )
                    for sub in range(2):
                        h = 2 * hp + sub
                        for nt in range(NT_O):
                            po_f = mmps.tile([P, 512], f32, tag="mm", name="mm")
                            po = po_f[:, :65]
                            for i in range(kp):
                                nc.tensor.matmul(
                                    po,
                                    lhsT=pT[
                                        :, 2 * i : 2 * i + 2, sub,
                                        nt * P : (nt + 1) * P,
                                    ],
                                    rhs=v_aug_h[:, 2 * i : 2 * i + 2, h, :],
                                    start=(i == 0), stop=(i == kp - 1 and not ktail),
                                    perf_mode=DR,
                                )
                            if ktail:
                                nc.tensor.matmul(
                                    po,
                                    lhsT=pT[:, NKT - 1, sub, nt * P : (nt + 1) * P],
                                    rhs=v_aug_h[:, NKT - 1, h, :],
                                    start=(kp == 0), stop=True,
                                )
                            rcp = lnp.tile([P, 1], f32, tag="rcp")
                            nc.vector.reciprocal(out=rcp, in_=po[:, 64:65])
                            nc.vector.tensor_scalar_mul(
                                out=o_sb[:, nt, h * HD : (h + 1) * HD],
                                in0=po[:, 0:HD], scalar1=rcp,
                            )

                # --- oT via xbar + cast ---
                _mark(nc, "oT")
                oT = big.tile([P, KC, 512], fp8, tag="hT")
                for nt in range(NT_O):
                    tt = ttmp.tile([P, KC, P], bf16, tag="tt")
                    nc.scalar.dma_start_transpose(out=tt, in_=o_sb[:, nt, :])
                    nc.gpsimd.tensor_copy(
                        out=oT[:, :, nt * P : (nt + 1) * P], in_=tt
                    )

                # --- proj (fp8 DR) + unscale + residual -> xmid ---
                _mark(nc, "proj")
                xmid = big.tile([P, NT_O, DIM], f32, tag="xmid")
                for nt in range(NT_O):
                    for n0, n1 in ((0, 512), (512, 768)):
                        ps_f = mmps.tile([P, 512], f32, tag="mm", name="mm")
                        ps = ps_f[:, : n1 - n0]
                        for j in range(pairs):
                            nc.tensor.matmul(
                                ps,
                                lhsT=oT[:, 2 * j : 2 * j + 2, nt * P : (nt + 1) * P],
                                rhs=wp_sb[:, 2 * j : 2 * j + 2, n0:n1],
                                start=(j == 0), stop=(j == pairs - 1),
                                perf_mode=DR,
                            )
                        nc.vector.scalar_tensor_tensor(
                            out=xmid[:, nt, n0:n1], in0=ps,
                            scalar=1.0 / (SW * SW), in1=xt_own[:, nt, n0:n1],
                            op0=ALU.mult, op1=ALU.add,
                        )
                        if "bp" in reps:
                            nc.gpsimd.tensor_add(
                                out=xmid[:, nt, n0:n1], in0=xmid[:, nt, n0:n1],
                                in1=reps["bp"][:, n0:n1],
                            )

                # --- LN2 + xbar transpose (+ cast if fc1 fp8) -> h2T ---
                _mark(nc, "ln2")
                h2T = big.tile([P, KC, 512], fp8 if fc1_fp8 else bf16, tag="h2T")
                mv2_g = lnp.tile([P, NT_O, 2], f32, tag="ln_mv")
                for nt in range(NT_O):
                    if nt % 2 == 0:
                        _ln_stats_act(nc, lnp, scr, xmid[:, nt, :], mv2_g, nt)
                    else:
                        _ln_stats(nc, lnp, xmid[:, nt, :], mv2_g, nt)
                rstd2_g = _ln_rstd_group(nc, lnp, mv2_g, eps_t, NT_O)
                for nt in range(NT_O):
                    h_t = htmp.tile([P, DIM], bf16, tag="h")
                    _ln_apply(
                        nc, xmid[:, nt, :], h_t, mv2_g, rstd2_g, nt,
                        reps.get("ln2g"), reps.get("ln2b"),
                    )
                    if fc1_fp8:
                        tt = ttmp.tile([P, KC, P], bf16, tag="tt")
                        nc.scalar.dma_start_transpose(out=tt, in_=h_t)
                        nc.gpsimd.tensor_copy(
                            out=h2T[:, :, nt * P : (nt + 1) * P], in_=tt
                        )
                    else:
                        nc.scalar.dma_start_transpose(
                            out=h2T[:, :, nt * P : (nt + 1) * P], in_=h_t
                        )

                # --- fc1 + gelu -> g1T ---
                _mark(nc, "fc1")
                g1T = big.tile([P, KH, 512], fp8 if fc2_fp8 else bf16, tag="g1T")
                gsc = 1.0 / SW if fc1_fp8 else 1.0
                for mt in range(KH):
                    ps = mmps.tile([P, 512], f32, tag="mm", name="mm")
                    if fc1_fp8:
                        for j in range(pairs):
                            nc.tensor.matmul(
                                ps,
                                lhsT=wf1_sb[
                                    :, 2 * j : 2 * j + 2, mt * P : (mt + 1) * P
                                ],
                                rhs=h2T[:, 2 * j : 2 * j + 2, :],
                                start=(j == 0), stop=(j == pairs - 1),
                                perf_mode=DR,
                            )
                    else:
                        for k in range(KC):
                            nc.tensor.matmul(
                                ps,
                                lhsT=wf1_sb[:, k, mt * P : (mt + 1) * P],
                                rhs=h2T[:, k, :],
                                start=(k == 0), stop=(k == KC - 1),
                            )
                    gl_bias = bf1_sb[:, mt : mt + 1] if bf1_sb is not None else 0.0
                    nc.scalar.activation(
                        out=g1T[:, mt, :], in_=ps, func=ACT_F.Gelu,
                        bias=gl_bias, scale=gsc,
                    )

                # --- fc2 + residual -> y ---
                _mark(nc, "fc2")
                y_sb = big.tile([P, NT_O, DIM], f32, tag="xt_own")
                y_r = y_e.rearrange("(t p) c -> p t c", p=P)
                kp2, ktail2 = _dr_steps(KH)
                for nt in range(NT_O):
                    for n0, n1 in ((0, 512), (512, 768)):
                        ps_f = mmps.tile([P, 512], f32, tag="mm", name="mm")
                        ps = ps_f[:, : n1 - n0]
                        if fc2_fp8:
                            for i in range(kp2):
                                nc.tensor.matmul(
                                    ps,
                                    lhsT=g1T[
                                        :, 2 * i : 2 * i + 2, nt * P : (nt + 1) * P
                                    ],
                                    rhs=wf2_sb[:, 2 * i : 2 * i + 2, n0:n1],
                                    start=(i == 0), stop=(i == kp2 - 1),
                                    perf_mode=DR,
                                )
                            nc.vector.scalar_tensor_tensor(
                                out=y_sb[:, nt, n0:n1], in0=ps, scalar=1.0 / SW,
                                in1=xmid[:, nt, n0:n1], op0=ALU.mult, op1=ALU.add,
                            )
                        else:
                            for k in range(KH):
                                nc.tensor.matmul(
                                    ps,
                                    lhsT=g1T[:, k, nt * P : (nt + 1) * P],
                                    rhs=wf2_sb[:, k, n0:n1],
                                    start=(k == 0), stop=(k == KH - 1),
                                )
                            nc.vector.tensor_add(
                                out=y_sb[:, nt, n0:n1], in0=ps,
                                in1=xmid[:, nt, n0:n1],
                            )
                        if "bf2" in reps:
                            nc.gpsimd.tensor_add(
                                out=y_sb[:, nt, n0:n1], in0=y_sb[:, nt, n0:n1],
                                in1=reps["bf2"][:, n0:n1],
                            )
                        nc.sync.dma_start(
                            out=y_r[:, nt, n0:n1], in_=y_sb[:, nt, n0:n1]
                        )

    nc.finalize()
    return nc


def _nontriv(a, val):
    return not np.allclose(np.asarray(a), val, rtol=0, atol=0)


_last_flags = None


def _prepare(x, attention_mask, ln1_g, ln1_b, ln2_g, ln2_b,
             w_qkv, b_qkv, w_proj, b_proj, w_fc1, b_fc1, w_fc2, b_fc2):
    x = np.ascontiguousarray(np.asarray(x, np.float32))
    attention_mask = np.asarray(attention_mask)
    B, N, C = x.shape
    H = N // 2  # 512

    kept = [np.nonzero(attention_mask[b] != 0)[0] for b in range(B)]
    maxk = max(len(k) for k in kept)
    assert maxk > 0, "all keys masked is not supported"
    NKT = -(-maxk // P)
    NK = NKT * P

    flags = {
        "ln1_gb": _nontriv(ln1_g, 1.0) or _nontriv(ln1_b, 0.0),
        "ln2_gb": _nontriv(ln2_g, 1.0) or _nontriv(ln2_b, 0.0),
        "bqk": _nontriv(b_qkv[: 2 * DIM], 0.0),
        "bv": _nontriv(b_qkv[2 * DIM :], 0.0),
        "bp": _nontriv(b_proj, 0.0),
        "bf1": _nontriv(b_fc1, 0.0),
        "bf2": _nontriv(b_fc2, 0.0),
        "nkt": NKT,
        "fc1_fp8": False,
        "fc2_fp8": True,
    }

    e4 = ml_dtypes.float8_e4m3fn
    w_qkv = np.asarray(w_qkv, np.float32)
    wqk = np.ascontiguousarray(w_qkv[:, : 2 * DIM] * SW).astype(e4)
    wv = np.ascontiguousarray(w_qkv[:, 2 * DIM :] * SW).astype(e4)
    wp = (np.asarray(w_proj, np.float32) * SW).astype(e4)
    if flags["fc1_fp8"]:
        wf1 = (np.asarray(w_fc1, np.float32) * SW).astype(e4)
    else:
        wf1 = np.asarray(w_fc1, np.float32).astype(ml_dtypes.bfloat16)
    if flags["fc2_fp8"]:
        wf2 = (np.asarray(w_fc2, np.float32) * SW).astype(e4)
    else:
        wf2 = np.asarray(w_fc2, np.float32).astype(ml_dtypes.bfloat16)

    shared = {"wqk": wqk, "wv": wv, "wp": wp, "wf1": wf1, "wf2": wf2}
    if flags["ln1_gb"]:
        shared["ln1g"] = np.asarray(ln1_g, np.float32)
        shared["ln1b"] = np.asarray(ln1_b, np.float32)
    if flags["ln2_gb"]:
        shared["ln2g"] = np.asarray(ln2_g, np.float32)
        shared["ln2b"] = np.asarray(ln2_b, np.float32)
    if flags["bqk"]:
        shared["bqk"] = np.asarray(b_qkv[: 2 * DIM], np.float32) * SW
    if flags["bv"]:
        shared["bv"] = np.asarray(b_qkv[2 * DIM :], np.float32) * SW
    if flags["bp"]:
        shared["bp"] = np.asarray(b_proj, np.float32)
    if flags["bf1"]:
        shared["bf1"] = np.asarray(b_fc1, np.float32)
    if flags["bf2"]:
        shared["bf2"] = np.asarray(b_fc2, np.float32)

    per_batch = []
    for b in range(B):
        idx = kept[b]
        xkb = np.zeros((NK, C), np.float32)
        xkb[: len(idx)] = x[b, idx]
        m = np.zeros((NK,), np.float32)
        m[: len(idx)] = 1.0
        per_batch.append(
            (
                np.ascontiguousarray(xkb).astype(ml_dtypes.bfloat16),
                np.ascontiguousarray(m.reshape(NKT, P).T),
            )
        )

    in_maps = []
    for c in range(N_CORES):
        b, hf = divmod(c, 2)
        xk_b, m01k_b = per_batch[b]
        in_maps.append(
            {
                "xp": np.ascontiguousarray(x[b, hf * H : (hf + 1) * H]),
                "xk": xk_b,
                "m01k": m01k_b,
                **shared,
            }
        )

    global _last_flags
    _last_flags = flags
    nc = _build(flags)
    return nc, in_maps, (B, N, C)


def kernel(**inputs):
    nc, in_maps, (B, N, C) = _prepare(**inputs)
    res = run_bass_kernel_spmd(nc, in_maps, list(range(N_CORES)))
    out = np.empty((B, N, C), np.float32)
    H = N // 2
    for c in range(N_CORES):
        b, hf = divmod(c, 2)
        out[b, hf * H : (hf + 1) * H] = res.results[c]["y"]
    return out
